# revision 9
# baseline (speedup 1.0000x reference)
"""Sigmoid-attention (DiffAttention) kernel for 8 Trainium2 NeuronCores.

Problem:  N=L=4096, H=8 heads, M=D=64.
    scores[n,l,h] = sigmoid(q[n,h,:] . k[l,h,:])
    out[n,h,:]    = (scores @ v) / sum_l(scores)        (per head)

Sharding: one head per core (8 heads == 8 cores). Each core gets its
head's Q/K transposed to [64, 4096] (duplicated onto both SBUF
partition halves) plus V packed as [V | ones] tiles, computes the full
attention for that head, and returns the head output transposed
([64, 4096] fp16); the host restores [4096, 8, 64] fp32.

The kernel is ACT-bound: sigmoid over 16.8M scores costs ~1ns/elem on
the Scalar engine (~131us); every other engine has slack. The PE runs
two tile-disjoint contraction-64 matmuls concurrently (T0 = partitions
0-63, T8 = 64-127), so all matmuls use 64-contraction halves:
    S^T[l,n]   = matmul(lhsT=K^T[:,l_tile], rhs=Q^T[:,n_chunk])
                 even l_tiles on T0, odd on T8 into one [128,1024]
                 PSUM tile (both halves of an l-tile pair)
    A^T[l,n]   = one sigmoid per pair  fp32 PSUM -> fp16 SBUF  (ACT)
    acc_a     += matmul(lhsT=[V|1][l 0:64],   rhs=A^T[0:64])    (T0)
    acc_b     += matmul(lhsT=[V|1][l 64:128], rhs=A^T[64:128])  (T8)
    out        = (acc_a+acc_b)[0:64] / (acc_a+acc_b)[64]  (DVE+GpSimd)

Scheduling keeps ACT saturated: one flat pipeline over all (chunk,
l-pair) steps with the mm1+sigmoid stream running LEAD pair-steps
ahead of the mm2 stream ACROSS chunk boundaries (no per-chunk drain
bubble), 3 PSUM score buffers so mm1 runs ahead of the sigmoid, and
input DMA ordered so the first l_tiles/chunk land first.
"""

from contextlib import ExitStack

import numpy as np

import concourse.bass as bass
import concourse.mybir as mybir
import concourse.tile as tile
from concourse import bacc
from concourse.bass import ts
from concourse.bass_utils import run_bass_kernel_spmd

N, L, H, M, D = 4096, 4096, 8, 64, 64
NCORES = 8
NCHUNK = 512  # n columns per PSUM accumulation chunk
NCHUNKS = N // NCHUNK
LTILES = L // 128
LPAIRS = LTILES // 2
VW = D + 1  # V columns + ones column
LEAD = 2  # mm1+sigmoid stream leads mm2 stream by LEAD pair-steps
CDT = mybir.dt.float16  # PE input dtype
FP32 = mybir.dt.float32
SIGMOID = mybir.ActivationFunctionType.Sigmoid

_CACHE: dict = {}


def build_nc():
    nc = bacc.Bacc("TRN2", target_bir_lowering=False, debug=False)

    q2_d = nc.dram_tensor("q2", [128, N], CDT, kind="ExternalInput").ap()
    k2_d = nc.dram_tensor("k2", [128, L], CDT, kind="ExternalInput").ap()
    v1_d = nc.dram_tensor("v1", [128, LTILES * VW], CDT, kind="ExternalInput").ap()
    out_d = nc.dram_tensor("out", [D, N], CDT, kind="ExternalOutput").ap()

    with ExitStack() as ctx:
        tc = ctx.enter_context(tile.TileContext(nc))
        const = ctx.enter_context(tc.tile_pool(name="const", bufs=1))
        apool = ctx.enter_context(tc.tile_pool(name="apool", bufs=LEAD + 2))
        io = ctx.enter_context(tc.tile_pool(name="io", bufs=3))
        psA = ctx.enter_context(tc.tile_pool(name="psA", bufs=3, space="PSUM"))
        psAcc = ctx.enter_context(tc.tile_pool(name="psAcc", bufs=1, space="PSUM"))

        # Input loads in 512-col slices, ordered so compute starts ASAP: the
        # first k/q slices (first l_tiles of chunk 0), then k/v (chunk 0
        # touches every l_tile), then the remaining q chunks.
        q2_s = const.tile([128, N], CDT)
        k2_s = const.tile([128, L], CDT)
        v1_s = const.tile([128, LTILES * VW], CDT)
        SL = 512
        nc.sync.dma_start(out=k2_s[:, 0:256], in_=k2_d[:, 0:256])
        nc.sync.dma_start(out=q2_s[:, 0:SL], in_=q2_d[:, 0:SL])
        nc.sync.dma_start(out=k2_s[:, 256:512], in_=k2_d[:, 256:512])
        for si in range(N // SL):
            ss = si * SL
            if si > 0:
                nc.sync.dma_start(out=k2_s[:, ss : ss + SL], in_=k2_d[:, ss : ss + SL])
            vs = si * 4 * VW
            nc.sync.dma_start(
                out=v1_s[:, vs : vs + 4 * VW], in_=v1_d[:, vs : vs + 4 * VW]
            )
        for si in range(1, N // SL):
            ss = si * SL
            nc.sync.dma_start(out=q2_s[:, ss : ss + SL], in_=q2_d[:, ss : ss + SL])

        def mm1pair(ci, lt, sT):
            # even l_tile on T0 (partitions 0-63) -> sT[:, 0:512], odd on
            # T8 (64-127) -> sT[:, 512:1024]; the two stream concurrently.
            cs = ci * NCHUNK
            ke = k2_s[0:64, ts(lt, 128)]
            ko = k2_s[64:128, ts(lt + 1, 128)]
            qsl = slice(cs, cs + NCHUNK)
            nc.tensor.matmul(sT[:, 0:NCHUNK], ke, q2_s[0:64, qsl], start=True, stop=True)
            nc.tensor.matmul(
                sT[:, NCHUNK : 2 * NCHUNK], ko, q2_s[64:128, qsl], start=True, stop=True
            )

        def mm2(lt, aT, acc_a, acc_b):
            # aT holds the pair (lt0, lt0+1); col offset selects the tile.
            off = (lt % 2) * NCHUNK
            asl = slice(off, off + NCHUNK)
            va = v1_s[0:64, lt * VW : (lt + 1) * VW]
            vb = v1_s[64:128, lt * VW : (lt + 1) * VW]
            first, last = lt == 0, lt == LTILES - 1
            nc.tensor.matmul(acc_a, va, aT[0:64, asl], start=first, stop=last)
            nc.tensor.matmul(acc_b, vb, aT[64:128, asl], start=first, stop=last)

        def epilogue(ci, acc_a, acc_b):
            # out[:, chunk] = sum[0:D] / sum[D] (normalizer row), where
            # sum = acc_a + acc_b. Two parallel branches (GpSimd broadcast
            # of the normalizer || DVE sum), then one multiply.
            cs = ci * NCHUNK
            tmp = io.tile([VW, NCHUNK], FP32, tag="tmp")
            nc.vector.tensor_copy(tmp, acc_a)
            norm_p = io.tile([1, NCHUNK], FP32, tag="norm")
            nc.vector.tensor_add(norm_p, tmp[D : D + 1, :], acc_b[D : D + 1, :])
            s64 = io.tile([D, NCHUNK], FP32, tag="s64")
            nc.vector.tensor_add(s64, tmp[0:D, :], acc_b[0:D, :])
            bc = io.tile([D, NCHUNK], FP32, tag="bc")
            nc.gpsimd.partition_broadcast(bc, norm_p, channels=D)
            rec = io.tile([D, NCHUNK], FP32, tag="rec")
            nc.vector.reciprocal_approx_fast(out=rec, in_=bc)
            o = io.tile([D, NCHUNK], CDT, tag="o")
            nc.vector.tensor_mul(o, s64, rec)
            nc.sync.dma_start(out=out_d[:, cs : cs + NCHUNK], in_=o)

        # Flat software pipeline over all (chunk, l-pair) steps.
        steps = [(ci, lt) for ci in range(NCHUNKS) for lt in range(0, LTILES, 2)]
        aTs: dict = {}
        accs: dict = {}

        def mm1sig(ci, lt):
            sT = psA.tile([128, 2 * NCHUNK], FP32, tag="sT", name="sT")
            mm1pair(ci, lt, sT)
            aT = apool.tile([128, 2 * NCHUNK], CDT, tag="aT", name="aT")
            aTs[(ci, lt)] = aT
            nc.scalar.activation(aT, sT, SIGMOID)

        for step in range(len(steps) + LEAD):
            # mm2 stream first: its sigmoid is LEAD steps old, so PE never
            # stalls here; any wait lands on mm1 (sT slot).
            if step >= LEAD:
                ci, lt = steps[step - LEAD]
                if lt == 0:
                    accs[ci] = (
                        psAcc.tile([VW, NCHUNK], FP32, tag="acc_a", name="acc_a"),
                        psAcc.tile([VW, NCHUNK], FP32, tag="acc_b", name="acc_b"),
                    )
                acc_a, acc_b = accs[ci]
                aT = aTs.pop((ci, lt))
                mm2(lt, aT, acc_a, acc_b)
                mm2(lt + 1, aT, acc_a, acc_b)
                if lt == LTILES - 2:
                    epilogue(ci, acc_a, acc_b)
                    del accs[ci]
            if step < len(steps):
                mm1sig(*steps[step])

    nc.compile()
    return nc


def get_nc():
    if "nc" not in _CACHE:
        _CACHE["nc"] = build_nc()
    return _CACHE["nc"]


def make_in_maps(queries, keys, values):
    np_cdt = mybir.dt.np(CDT)
    in_maps = []
    for h in range(NCORES):
        qT = np.ascontiguousarray(queries[:, h, :].T.astype(np_cdt))
        kT = np.ascontiguousarray(keys[:, h, :].T.astype(np_cdt))
        v1 = np.empty((L, VW), np_cdt)
        v1[:, :D] = values[:, h, :]
        v1[:, D] = 1.0
        v1p = np.ascontiguousarray(
            v1.reshape(LTILES, 128, VW).transpose(1, 0, 2).reshape(128, LTILES * VW)
        )
        in_maps.append(
            {
                "q2": np.vstack([qT, qT]),
                "k2": np.vstack([kT, kT]),
                "v1": v1p,
            }
        )
    return in_maps


def run(queries, keys, values, trace=False):
    """Returns (out [N,H,D] fp32, BassKernelResults)."""
    nc = get_nc()
    in_maps = make_in_maps(queries, keys, values)
    res = run_bass_kernel_spmd(nc, in_maps, core_ids=list(range(NCORES)), trace=trace)
    out = np.empty((N, H, D), np.float32)
    for h in range(NCORES):
        out[:, h, :] = res.results[h]["out"].T.astype(np.float32)
    return out, res


def kernel(queries, keys, values):
    out, _ = run(np.asarray(queries), np.asarray(keys), np.asarray(values))
    return out


# revision 15
# speedup vs baseline: 1.0622x; 1.0622x over previous
"""Sigmoid-attention (DiffAttention) kernel for 8 Trainium2 NeuronCores.

Problem:  N=L=4096, H=8 heads, M=D=64.
    scores[n,l,h] = sigmoid(q[n,h,:] . k[l,h,:])
    out[n,h,:]    = (scores @ v) / sum_l(scores)        (per head)

Sharding: one head per core (8 heads == 8 cores). Each core gets its
head's Q/K transposed to [64, 4096] (duplicated onto both SBUF
partition halves) plus V packed as [V | ones] tiles, computes the full
attention for that head, and returns the head output transposed
([64, 4096] fp16); the host restores [4096, 8, 64] fp32.

The kernel is ACT-bound: sigmoid over 16.8M scores costs ~1ns/elem on
the Scalar engine (~131us); every other engine has slack. The PE runs
two tile-disjoint contraction-64 matmuls concurrently (T0 = partitions
0-63, T8 = 64-127), so all matmuls use 64-contraction halves:
    S^T[l,n]   = matmul(lhsT=K^T[:,l_tile], rhs=Q^T[:,n_chunk])
                 even l_tiles on T0, odd on T8 into one [128,1024]
                 PSUM tile (both halves of an l-tile pair)
    A^T[l,n]   = one sigmoid per pair  fp32 PSUM -> fp16 SBUF  (ACT)
    acc_a     += matmul(lhsT=[V|1][l 0:64],   rhs=A^T[0:64])    (T0)
    acc_b     += matmul(lhsT=[V|1][l 64:128], rhs=A^T[64:128])  (T8)
    out        = (acc_a+acc_b)[0:64] / (acc_a+acc_b)[64]  (DVE+GpSimd)

Scheduling keeps ACT saturated: one flat pipeline over all (chunk,
l-pair) steps with the mm1+sigmoid stream running LEAD pair-steps
ahead of the mm2 stream ACROSS chunk boundaries (no per-chunk drain
bubble), 3 PSUM score buffers so mm1 runs ahead of the sigmoid, and
input DMA ordered so the first l_tiles/chunk land first.
"""

from contextlib import ExitStack

import numpy as np

import concourse.bass as bass
import concourse.mybir as mybir
import concourse.tile as tile
from concourse import bacc
from concourse.bass import ts
from concourse.bass_utils import run_bass_kernel_spmd

N, L, H, M, D = 4096, 4096, 8, 64, 64
NCORES = 8
NCHUNK = 512  # n columns per PSUM accumulation chunk
NCHUNKS = N // NCHUNK
LTILES = L // 128
LPAIRS = LTILES // 2
VW = D + 1  # V columns + ones column
LEAD = 2  # mm1+sigmoid stream leads mm2 stream by LEAD pair-steps
CDT = mybir.dt.float16  # PE input dtype
FP32 = mybir.dt.float32
SIGMOID = mybir.ActivationFunctionType.Sigmoid

_CACHE: dict = {}


def build_nc():
    nc = bacc.Bacc("TRN2", target_bir_lowering=False, debug=False)

    q2_d = nc.dram_tensor("q2", [128, N], CDT, kind="ExternalInput").ap()
    k2_d = nc.dram_tensor("k2", [128, L // 2], CDT, kind="ExternalInput").ap()
    v1_d = nc.dram_tensor("v1", [128, LTILES * VW], CDT, kind="ExternalInput").ap()
    out_d = nc.dram_tensor("out", [D, N], CDT, kind="ExternalOutput").ap()

    with ExitStack() as ctx:
        tc = ctx.enter_context(tile.TileContext(nc))
        const = ctx.enter_context(tc.tile_pool(name="const", bufs=1))
        apool = ctx.enter_context(tc.tile_pool(name="apool", bufs=LEAD + 2))
        io = ctx.enter_context(tc.tile_pool(name="io", bufs=3))
        psA = ctx.enter_context(tc.tile_pool(name="psA", bufs=3, space="PSUM"))
        psAcc = ctx.enter_context(tc.tile_pool(name="psAcc", bufs=1, space="PSUM"))

        # Input loads in 512-col slices, ordered so compute starts ASAP: the
        # first k/q slices (first l_tiles of chunk 0), then k/v (chunk 0
        # touches every l_tile), then the remaining q chunks.
        q2_s = const.tile([128, N], CDT)
        k2_s = const.tile([128, L // 2], CDT)
        v1_s = const.tile([128, LTILES * VW], CDT)
        SL = 512
        nc.sync.dma_start(out=k2_s[:, 0:256], in_=k2_d[:, 0:256])
        nc.sync.dma_start(out=q2_s[:, 0:SL], in_=q2_d[:, 0:SL])
        nc.sync.dma_start(out=k2_s[:, 256:512], in_=k2_d[:, 256:512])
        for si in range(N // SL):
            ss = si * SL
            if 0 < si < 4:
                nc.sync.dma_start(out=k2_s[:, ss : ss + SL], in_=k2_d[:, ss : ss + SL])
            vs = si * 4 * VW
            nc.sync.dma_start(
                out=v1_s[:, vs : vs + 4 * VW], in_=v1_d[:, vs : vs + 4 * VW]
            )
        for si in range(1, N // SL):
            ss = si * SL
            nc.sync.dma_start(out=q2_s[:, ss : ss + SL], in_=q2_d[:, ss : ss + SL])

        def mm1pair(ci, lt, sT):
            # even l_tile on T0 (partitions 0-63) -> sT[:, 0:512], odd on
            # T8 (64-127) -> sT[:, 512:1024]; the two stream concurrently.
            # k2 is packed [evens; odds]: both halves read the same columns.
            cs = ci * NCHUNK
            ke = k2_s[0:64, ts(lt // 2, 128)]
            ko = k2_s[64:128, ts(lt // 2, 128)]
            qsl = slice(cs, cs + NCHUNK)
            nc.tensor.matmul(sT[:, 0:NCHUNK], ke, q2_s[0:64, qsl], start=True, stop=True)
            nc.tensor.matmul(
                sT[:, NCHUNK : 2 * NCHUNK], ko, q2_s[64:128, qsl], start=True, stop=True
            )

        def mm2(lt, aT, acc_a, acc_b):
            # aT holds the pair (lt0, lt0+1); col offset selects the tile.
            off = (lt % 2) * NCHUNK
            asl = slice(off, off + NCHUNK)
            va = v1_s[0:64, lt * VW : (lt + 1) * VW]
            vb = v1_s[64:128, lt * VW : (lt + 1) * VW]
            first, last = lt == 0, lt == LTILES - 1
            nc.tensor.matmul(acc_a, va, aT[0:64, asl], start=first, stop=last)
            nc.tensor.matmul(acc_b, vb, aT[64:128, asl], start=first, stop=last)

        def epilogue(ci, acc_a, acc_b):
            # out[:, chunk] = sum[0:D] / sum[D] (normalizer row), where
            # sum = acc_a + acc_b. Two parallel branches (GpSimd broadcast
            # of the normalizer || DVE sum), then one multiply.
            cs = ci * NCHUNK
            tmp = io.tile([VW, NCHUNK], FP32, tag="tmp")
            nc.vector.tensor_copy(tmp, acc_a)
            norm_p = io.tile([1, NCHUNK], FP32, tag="norm")
            nc.vector.tensor_add(norm_p, tmp[D : D + 1, :], acc_b[D : D + 1, :])
            s64 = io.tile([D, NCHUNK], FP32, tag="s64")
            nc.vector.tensor_add(s64, tmp[0:D, :], acc_b[0:D, :])
            bc = io.tile([D, NCHUNK], FP32, tag="bc")
            nc.gpsimd.partition_broadcast(bc, norm_p, channels=D)
            rec = io.tile([D, NCHUNK], FP32, tag="rec")
            nc.vector.reciprocal_approx_fast(out=rec, in_=bc)
            o = io.tile([D, NCHUNK], CDT, tag="o")
            nc.vector.tensor_mul(o, s64, rec)
            nc.sync.dma_start(out=out_d[:, cs : cs + NCHUNK], in_=o)

        # Flat software pipeline over all (chunk, l-pair) steps.
        steps = [(ci, lt) for ci in range(NCHUNKS) for lt in range(0, LTILES, 2)]
        aTs: dict = {}
        accs: dict = {}

        def mm1sig(ci, lt):
            sT = psA.tile([128, 2 * NCHUNK], FP32, tag="sT", name="sT")
            mm1pair(ci, lt, sT)
            aT = apool.tile([128, 2 * NCHUNK], CDT, tag="aT", name="aT")
            aTs[(ci, lt)] = aT
            nc.scalar.activation(aT, sT, SIGMOID)

        for step in range(len(steps) + LEAD):
            # mm1+sigmoid stream first: with 3 sT buffers mm1's own wait
            # (sT slot) has ~2 sigmoid-periods of slack, and issuing it
            # ahead of mm2 keeps ACT fed even when mm2 stalls on the
            # accumulator WAR against the previous chunk's epilogue.
            if step < len(steps):
                mm1sig(*steps[step])
            if step >= LEAD:
                ci, lt = steps[step - LEAD]
                if lt == 0:
                    accs[ci] = (
                        psAcc.tile([VW, NCHUNK], FP32, tag="acc_a", name="acc_a"),
                        psAcc.tile([VW, NCHUNK], FP32, tag="acc_b", name="acc_b"),
                    )
                acc_a, acc_b = accs[ci]
                aT = aTs.pop((ci, lt))
                mm2(lt, aT, acc_a, acc_b)
                mm2(lt + 1, aT, acc_a, acc_b)
                if lt == LTILES - 2:
                    epilogue(ci, acc_a, acc_b)
                    del accs[ci]

    nc.compile()
    return nc


def get_nc():
    if "nc" not in _CACHE:
        _CACHE["nc"] = build_nc()
    return _CACHE["nc"]


def make_in_maps(queries, keys, values):
    np_cdt = mybir.dt.np(CDT)
    in_maps = []
    for h in range(NCORES):
        qT = np.ascontiguousarray(queries[:, h, :].T.astype(np_cdt))
        kT = keys[:, h, :].T.astype(np_cdt)  # [64, L]
        kt3 = kT.reshape(M, LTILES, 128)
        k2p = np.vstack(
            [
                kt3[:, 0::2, :].reshape(M, L // 2),
                kt3[:, 1::2, :].reshape(M, L // 2),
            ]
        )
        v1 = np.empty((L, VW), np_cdt)
        v1[:, :D] = values[:, h, :]
        v1[:, D] = 1.0
        v1p = np.ascontiguousarray(
            v1.reshape(LTILES, 128, VW).transpose(1, 0, 2).reshape(128, LTILES * VW)
        )
        in_maps.append(
            {
                "q2": np.vstack([qT, qT]),
                "k2": np.ascontiguousarray(k2p),
                "v1": v1p,
            }
        )
    return in_maps


def run(queries, keys, values, trace=False):
    """Returns (out [N,H,D] fp32, BassKernelResults)."""
    nc = get_nc()
    in_maps = make_in_maps(queries, keys, values)
    res = run_bass_kernel_spmd(nc, in_maps, core_ids=list(range(NCORES)), trace=trace)
    out = np.empty((N, H, D), np.float32)
    for h in range(NCORES):
        out[:, h, :] = res.results[h]["out"].T.astype(np.float32)
    return out, res


def kernel(queries, keys, values):
    out, _ = run(np.asarray(queries), np.asarray(keys), np.asarray(values))
    return out


# revision 20
# speedup vs baseline: 1.1029x; 1.0383x over previous
"""Sigmoid-attention (DiffAttention) kernel for 8 Trainium2 NeuronCores.

Problem:  N=L=4096, H=8 heads, M=D=64.
    scores[n,l,h] = sigmoid(q[n,h,:] . k[l,h,:])
    out[n,h,:]    = (scores @ v) / sum_l(scores)        (per head)

Sharding: one head per core (8 heads == 8 cores). Each core gets its
head's Q/K transposed to [64, 4096] (duplicated onto both SBUF
partition halves) plus V packed as [V | ones] tiles, computes the full
attention for that head, and returns the head output transposed
([64, 4096] fp16); the host restores [4096, 8, 64] fp32.

The kernel is ACT-bound: sigmoid over 16.8M scores costs ~1ns/elem on
the Scalar engine (~131us); every other engine has slack. The PE runs
two tile-disjoint contraction-64 matmuls concurrently (T0 = partitions
0-63, T8 = 64-127), so all matmuls use 64-contraction halves:
    S^T[l,n]   = matmul(lhsT=K^T[:,l_tile], rhs=Q^T[:,n_chunk])
                 even l_tiles on T0, odd on T8 into one [128,1024]
                 PSUM tile (both halves of an l-tile pair)
    A^T[l,n]   = one sigmoid per pair  fp32 PSUM -> fp16 SBUF  (ACT)
    acc_a     += matmul(lhsT=[V|1][l 0:64],   rhs=A^T[0:64])    (T0)
    acc_b     += matmul(lhsT=[V|1][l 64:128], rhs=A^T[64:128])  (T8)
    out        = (acc_a+acc_b)[0:64] / (acc_a+acc_b)[64]  (DVE+GpSimd)

Scheduling keeps ACT saturated: one flat pipeline over all (chunk,
l-pair) steps with the mm1+sigmoid stream running LEAD pair-steps
ahead of the mm2 stream ACROSS chunk boundaries (no per-chunk drain
bubble), 3 PSUM score buffers so mm1 runs ahead of the sigmoid, and
input DMA ordered so the first l_tiles/chunk land first.
"""

from contextlib import ExitStack

import numpy as np

import concourse.bass as bass
import concourse.mybir as mybir
import concourse.tile as tile
from concourse import bacc
from concourse.bass import ts
from concourse.bass_utils import run_bass_kernel_spmd

N, L, H, M, D = 4096, 4096, 8, 64, 64
NCORES = 8
NCHUNK = 512  # n columns per PSUM accumulation chunk
NCHUNKS = N // NCHUNK
LTILES = L // 128
GRP = 3  # l_tiles per score buffer / sigmoid instruction (last group is 2)
VW = D + 1  # V columns + ones column
LEAD = 2  # mm1+sigmoid stream leads mm2 stream by LEAD group-steps
CDT = mybir.dt.float16  # PE input dtype
FP32 = mybir.dt.float32
SIGMOID = mybir.ActivationFunctionType.Sigmoid

_CACHE: dict = {}


def build_nc():
    nc = bacc.Bacc("TRN2", target_bir_lowering=False, debug=False)

    q2_d = nc.dram_tensor("q2", [128, N], CDT, kind="ExternalInput").ap()
    k2_d = nc.dram_tensor("k2", [128, L // 2], CDT, kind="ExternalInput").ap()
    v1_d = nc.dram_tensor("v1", [128, LTILES * VW], CDT, kind="ExternalInput").ap()
    out_d = nc.dram_tensor("out", [D, N], CDT, kind="ExternalOutput").ap()

    with ExitStack() as ctx:
        tc = ctx.enter_context(tile.TileContext(nc))
        const = ctx.enter_context(tc.tile_pool(name="const", bufs=1))
        apool = ctx.enter_context(tc.tile_pool(name="apool", bufs=LEAD + 2))
        io = ctx.enter_context(tc.tile_pool(name="io", bufs=3))
        psA = ctx.enter_context(tc.tile_pool(name="psA", bufs=2, space="PSUM"))
        psAcc = ctx.enter_context(tc.tile_pool(name="psAcc", bufs=1, space="PSUM"))

        # Input loads in 512-col slices, ordered so compute starts ASAP: the
        # first k/q slices (first l_tiles of chunk 0), then k/v (chunk 0
        # touches every l_tile), then the remaining q chunks.
        q2_s = const.tile([128, N], CDT)
        k2_s = const.tile([128, L // 2], CDT)
        v1_s = const.tile([128, LTILES * VW], CDT)
        SL = 512
        nc.sync.dma_start(out=k2_s[:, 0:256], in_=k2_d[:, 0:256])
        nc.sync.dma_start(out=q2_s[:, 0:SL], in_=q2_d[:, 0:SL])
        nc.sync.dma_start(out=k2_s[:, 256:512], in_=k2_d[:, 256:512])
        for si in range(N // SL):
            ss = si * SL
            if 0 < si < 4:
                nc.sync.dma_start(out=k2_s[:, ss : ss + SL], in_=k2_d[:, ss : ss + SL])
            vs = si * 4 * VW
            nc.sync.dma_start(
                out=v1_s[:, vs : vs + 4 * VW], in_=v1_d[:, vs : vs + 4 * VW]
            )
        for si in range(1, N // SL):
            ss = si * SL
            nc.sync.dma_start(out=q2_s[:, ss : ss + SL], in_=q2_d[:, ss : ss + SL])

        def mm1group(ci, lt0, g, sT):
            # l_tile t runs on half t%2 (T0 = partitions 0-63, T8 = 64-127);
            # the halves stream concurrently. k2 is packed [evens; odds], so
            # both halves read column block t//2.
            cs = ci * NCHUNK
            qsl = slice(cs, cs + NCHUNK)
            for j in range(g):
                t = lt0 + j
                hp = 64 * (t % 2)
                nc.tensor.matmul(
                    sT[:, ts(j, NCHUNK)],
                    k2_s[hp : hp + 64, ts(t // 2, 128)],
                    q2_s[hp : hp + 64, qsl],
                    start=True,
                    stop=True,
                )

        def mm2group(lt0, g, aT, acc_a, acc_b):
            for j in range(g):
                t = lt0 + j
                asl = ts(j, NCHUNK)
                va = v1_s[0:64, t * VW : (t + 1) * VW]
                vb = v1_s[64:128, t * VW : (t + 1) * VW]
                first, last = t == 0, t == LTILES - 1
                nc.tensor.matmul(acc_a, va, aT[0:64, asl], start=first, stop=last)
                nc.tensor.matmul(acc_b, vb, aT[64:128, asl], start=first, stop=last)

        def epilogue(ci, acc_a, acc_b, split=1):
            # out[:, chunk] = sum[0:D] / sum[D] (normalizer row), where
            # sum = acc_a + acc_b. Two parallel branches (GpSimd broadcast
            # of the normalizer || DVE sum), then one multiply. The last
            # chunk runs in halves so the stages pipeline across engines.
            cs = ci * NCHUNK
            w = NCHUNK // split
            for h in range(split):
                hs = slice(h * w, (h + 1) * w)
                tmp = io.tile([VW, NCHUNK], FP32, tag="tmp", name="tmp")
                nc.vector.tensor_copy(tmp[:, 0:w], acc_a[:, hs])
                norm_p = io.tile([1, NCHUNK], FP32, tag="norm", name="norm_p")
                nc.vector.tensor_add(
                    norm_p[:, 0:w], tmp[D : D + 1, 0:w], acc_b[D : D + 1, hs]
                )
                s64 = io.tile([D, NCHUNK], FP32, tag="s64", name="s64")
                nc.vector.tensor_add(s64[:, 0:w], tmp[0:D, 0:w], acc_b[0:D, hs])
                bc = io.tile([D, NCHUNK], FP32, tag="bc", name="bc")
                nc.gpsimd.partition_broadcast(bc[:, 0:w], norm_p[:, 0:w], channels=D)
                rec = io.tile([D, NCHUNK], FP32, tag="rec", name="rec")
                nc.vector.reciprocal_approx_fast(out=rec[:, 0:w], in_=bc[:, 0:w])
                o = io.tile([D, NCHUNK], CDT, tag="o", name="o")
                nc.vector.tensor_mul(o[:, 0:w], s64[:, 0:w], rec[:, 0:w])
                nc.sync.dma_start(out=out_d[:, cs + h * w : cs + (h + 1) * w], in_=o[:, 0:w])

        # Flat software pipeline over all (chunk, l-tile-group) steps.
        steps = [
            (ci, lt0, min(GRP, LTILES - lt0))
            for ci in range(NCHUNKS)
            for lt0 in range(0, LTILES, GRP)
        ]
        aTs: dict = {}
        accs: dict = {}

        def mm1sig(ci, lt0, g):
            sT = psA.tile([128, GRP * NCHUNK], FP32, tag="sT", name="sT")
            mm1group(ci, lt0, g, sT)
            aT = apool.tile([128, GRP * NCHUNK], CDT, tag="aT", name="aT")
            aTs[(ci, lt0)] = aT
            nc.scalar.activation(aT[:, 0 : g * NCHUNK], sT[:, 0 : g * NCHUNK], SIGMOID)

        for step in range(len(steps) + LEAD):
            # mm1+sigmoid stream first: issuing it ahead of mm2 keeps ACT
            # fed even when mm2 stalls (e.g. on the accumulator WAR against
            # the previous chunk's epilogue); mm1's own wait (sT ping-pong
            # slot) has a full sigmoid-period of slack.
            if step < len(steps):
                mm1sig(*steps[step])
            if step >= LEAD:
                ci, lt0, g = steps[step - LEAD]
                if lt0 == 0:
                    accs[ci] = (
                        psAcc.tile([VW, NCHUNK], FP32, tag="acc_a", name="acc_a"),
                        psAcc.tile([VW, NCHUNK], FP32, tag="acc_b", name="acc_b"),
                    )
                acc_a, acc_b = accs[ci]
                aT = aTs.pop((ci, lt0))
                mm2group(lt0, g, aT, acc_a, acc_b)
                if lt0 + g == LTILES:
                    epilogue(ci, acc_a, acc_b)
                    del accs[ci]

    nc.compile()
    return nc


def get_nc():
    if "nc" not in _CACHE:
        _CACHE["nc"] = build_nc()
    return _CACHE["nc"]


def make_in_maps(queries, keys, values):
    np_cdt = mybir.dt.np(CDT)
    in_maps = []
    for h in range(NCORES):
        qT = np.ascontiguousarray(queries[:, h, :].T.astype(np_cdt))
        kT = keys[:, h, :].T.astype(np_cdt)  # [64, L]
        kt3 = kT.reshape(M, LTILES, 128)
        k2p = np.vstack(
            [
                kt3[:, 0::2, :].reshape(M, L // 2),
                kt3[:, 1::2, :].reshape(M, L // 2),
            ]
        )
        v1 = np.empty((L, VW), np_cdt)
        v1[:, :D] = values[:, h, :]
        v1[:, D] = 1.0
        v1p = np.ascontiguousarray(
            v1.reshape(LTILES, 128, VW).transpose(1, 0, 2).reshape(128, LTILES * VW)
        )
        in_maps.append(
            {
                "q2": np.vstack([qT, qT]),
                "k2": np.ascontiguousarray(k2p),
                "v1": v1p,
            }
        )
    return in_maps


def run(queries, keys, values, trace=False):
    """Returns (out [N,H,D] fp32, BassKernelResults)."""
    nc = get_nc()
    in_maps = make_in_maps(queries, keys, values)
    res = run_bass_kernel_spmd(nc, in_maps, core_ids=list(range(NCORES)), trace=trace)
    out = np.empty((N, H, D), np.float32)
    for h in range(NCORES):
        out[:, h, :] = res.results[h]["out"].T.astype(np.float32)
    return out, res


def kernel(queries, keys, values):
    out, _ = run(np.asarray(queries), np.asarray(keys), np.asarray(values))
    return out
